# revision 18
# baseline (speedup 1.0000x reference)
"""Expert-parallel MoE conditional feed-forward for 8 Trainium2 NeuronCores.

Problem: x[16,1024], expert_indices[16,2], gate/down_proj[8,2816,1024],
up_proj[8,1024,2816]. Reference computes, per (token, slot) pair with
e = expert_indices[t, a]:
    out[t,a,:] = (silu(x @ gate_proj[e].T) * (x @ down_proj[e].T)) @ up_proj[e].T

Sharding: core k owns expert k and computes its FFN output for ALL 16
tokens (the compute is negligible; the kernel is weight-streaming bound).
The host then gathers rows per expert_indices. This needs no indices on
device and is load-balanced regardless of routing.

Weights are streamed as bf16 (host-side downcast): fp32 PSUM accumulation
keeps the error ~1e-3, far inside the 2e-2 gate, and it HALVES the HBM
traffic (34.6 MB -> 17.3 MB per core, ~48 us roofline at 358 GB/s).
bf16 matmuls also run at 1 cycle/column on the PE (vs 4 for fp32), so
the PE is far from the bottleneck.

Device kernel (per core): loop over 11 chunks of 256 of the 2816-wide
intermediate dim. Weights are host-packed into W[11, 128, 6144]:
    W[c,p, hc*512+o]        o<256: gate block g[c*256+o, hc*128+p]
                            o>=256: down block d[c*256+o-256, hc*128+p]
    W[c,p, 4096+f*1024+j]   up block u[j, c*256+f*128+p]
ALL chunks live in one persistent SBUF tile (135 KB/partition) and every
chunk's DMA is issued up-front, back-to-back: the 16 DMA queues never
stall on buffer-recycle semaphores and stream at ~400 GB/s; per-chunk
instructions keep dependencies fine enough for the PE to chase the
stream. The first/last chunks are split finer (PE ramps sooner; the
drain chases descriptors), and the last two chunks' gate|down blocks are
hoisted before the trailing up blocks.

All big matmuls stream the WEIGHT as the moving operand (the stationary
is a 16-column token tile). Since only 16 of 128 array columns are used
(M = 16 tokens), we run THREE concurrent column quarter-strips (array
packing, tile inferred from PSUM base partition; q3 is unusable per HW
bug):
    q1 (psum rows 32-47): gate|down chains for even-position chunks
    q2 (psum rows 64-79): gate|down chains for odd-position chunks
    q0 (psum rows  0-15): all up-projection accumulation into psum_out
Chunks run in pairs ((0) alone, then (1,2)...(9,10) so the final drain
overlaps both strips); the previous pair's up-matmuls are round-robin
interleaved with the current pair's chain matmuls so consecutive PE
instructions hit different strips and overlap. The [16,128]
intermediates are transposed to [128,16] bf16 via PE transpose-mode
identity matmuls, then fed as stationaries. The two output halves are
copied/DMA'd on different engines (ACT and DVE/SP) as soon as their
accumulation chain stops, overlapping the final matmuls.
"""

import sys

for _p in ("/opt/trn_rl_repo", "/opt/pypackages"):
    if _p not in sys.path:
        sys.path.append(_p)

import numpy as np

NUM_EXPERTS = 8
HIDDEN = 1024
INTER = 2816
T = 16
N_CORES = 8
P = 128
CW = 256                  # intermediate chunk width
NCHUNK = INTER // CW      # 11
HC = HIDDEN // P          # 8 hidden chunks
U_OFF = 2 * HC * CW       # 4096: offset of up blocks in packed W
WCOLS = U_OFF + 2 * HIDDEN  # 6144

_COMPILED = None
LAST_RESULTS = None
TRACE = False


def _build():
    import concourse.bacc as bacc
    import concourse.bass as bass
    import concourse.tile as tile
    from concourse import mybir

    f32 = mybir.dt.float32
    bf16 = mybir.dt.bfloat16
    nc = bacc.Bacc("TRN2", target_bir_lowering=False, debug=False,
                   num_devices=N_CORES)
    xt_d = nc.dram_tensor("xt", [P, HC * T], bf16, kind="ExternalInput")
    eye_d = nc.dram_tensor("eye", [T, T], bf16, kind="ExternalInput")
    w_d = nc.dram_tensor("w", [NCHUNK, P, WCOLS], bf16, kind="ExternalInput")
    out_d = nc.dram_tensor("out", [T, HIDDEN], f32, kind="ExternalOutput")

    with tile.TileContext(nc) as tc:
        with (
            tc.tile_pool(name="xp", bufs=1) as xp,
            tc.tile_pool(name="wp", bufs=1) as wp,
            tc.tile_pool(name="ip", bufs=4) as ip,
            tc.tile_pool(name="pg", bufs=4, space=bass.MemorySpace.PSUM) as pgp,
            tc.tile_pool(name="tp", bufs=2, space=bass.MemorySpace.PSUM) as tpp,
            tc.tile_pool(name="po", bufs=1, space=bass.MemorySpace.PSUM) as pop,
            tc.tile_pool(name="op", bufs=1) as op,
        ):
            xt = xp.tile([P, HC * T], bf16)
            eye = xp.tile([T, T], bf16)

            # One persistent SBUF tile holds ALL weights (135 KB/partition).
            # Every chunk DMA is issued up-front, back-to-back: the queues
            # never stall on a buffer-recycle semaphore, and per-chunk
            # instructions keep the dependency granularity fine enough for
            # the PE to chase the stream. First/last chunks are split finer
            # so the PE starts sooner and the tail chases descriptors.
            wt = wp.tile([P, NCHUNK * WCOLS], bf16)

            # (chunk, lo, hi) in DMA issue order. First chunk split fine so
            # the PE starts early; the last chunk's gate|down blocks are
            # hoisted before the second-to-last chunk's up block so the
            # final gate/down chain starts ~1.5us sooner.
            sched = []
            for c in range(NCHUNK):
                if c == 0:
                    sched += [(c, 0, 512), ("xt",), ("eye",),
                              (c, 512, 1024), (c, 1024, 2048),
                              (c, 2048, 4096), (c, 4096, 6144)]
                elif c == 1:
                    sched += [(c, 0, 2048), (c, 2048, 4096), (c, 4096, 6144)]
                elif c == NCHUNK - 3:
                    # wu8 must precede gd9/gd10: chunk 8's deferred
                    # up-matmuls interleave into the (9,10) gate/down
                    # phase and the PE issues in order.
                    sched += [(c, 0, 4096), (c, 4096, 6144),
                              (NCHUNK - 2, 0, 4096),
                              (NCHUNK - 1, 0, 2048),
                              (NCHUNK - 1, 2048, 4096)]
                elif c == NCHUNK - 2:
                    sched += [(c, 4096, 6144)]
                elif c == NCHUNK - 1:
                    sched += [(c, 4096, 5120), (c, 5120, 6144)]
                else:
                    sched += [(c, 0, 6144)]
            for item in sched:
                if item == ("xt",):
                    nc.sync.dma_start(xt[:], xt_d.ap())
                elif item == ("eye",):
                    nc.sync.dma_start(eye[:], eye_d.ap())
                else:
                    c, lo, hi = item
                    nc.sync.dma_start(wt[:, c * WCOLS + lo:c * WCOLS + hi],
                                      w_d.ap()[c][:, lo:hi])

            psum_out = pop.tile([T, HIDDEN], f32)
            mm3_count = [0, 0]   # per-jb position in the accumulation chain
            pending_mm3 = []     # thunks deferred from the previous pair

            def emit_chunk_tail(c, pgd, base):
                """silu/mul + transposes for chunk c; queue its 4 up-matmuls."""
                ub = c * WCOLS + U_OFF
                s1 = ip.tile([T, CW], f32)
                nc.scalar.activation(s1[:], pgd[base:base + T, 0:CW],
                                     mybir.ActivationFunctionType.Silu)
                inter = ip.tile([T, CW], bf16)
                nc.vector.tensor_mul(inter[:], s1[:],
                                     pgd[base:base + T, CW:2 * CW])
                for f in range(CW // P):
                    tp = tpp.tile([P, T], bf16)
                    nc.tensor.transpose(tp[:], inter[:, f * P:(f + 1) * P],
                                        eye[:])
                    it = ip.tile([P, T], bf16)
                    nc.vector.tensor_copy(it[:], tp[:])
                    for jb in range(HIDDEN // 512):
                        def mm3(it=it, ub=ub, f=f, jb=jb):
                            k = mm3_count[jb]
                            mm3_count[jb] += 1
                            nc.tensor.matmul(
                                psum_out[:, jb * 512:(jb + 1) * 512], it[:],
                                wt[:, ub + f * HIDDEN + jb * 512:
                                    ub + f * HIDDEN + (jb + 1) * 512],
                                start=(k == 0), stop=(k == 2 * NCHUNK - 1),
                            )
                        pending_mm3.append(mm3)

            # Chunk 0 runs alone (it only needs the first DMA, so the PE
            # ramps without stalling on chunk 1's load); chunks 9+10 run
            # as the final pair so the drain overlaps on both strips.
            for pair in [(0,), (1, 2), (3, 4), (5, 6), (7, 8), (9, 10)]:
                tiles = []
                for c, base in zip(pair, (32, 64)):
                    pgd = pgp.tile([P, 2 * CW], f32)
                    tiles.append((c, pgd, base))

                todo = pending_mm3
                pending_mm3 = []
                for hc in range(HC):
                    for c, pgd, base in tiles:
                        gb = c * WCOLS + hc * 2 * CW
                        nc.tensor.matmul(
                            pgd[base:base + T, :], xt[:, hc * T:(hc + 1) * T],
                            wt[:, gb:gb + 2 * CW],
                            start=(hc == 0), stop=(hc == HC - 1),
                        )
                    if todo:
                        todo.pop(0)()
                while todo:
                    todo.pop(0)()

                for c, pgd, base in tiles:
                    emit_chunk_tail(c, pgd, base)

            # Finish each jb's accumulation chain first, then immediately
            # copy that half out (scalar engine for jb0, vector for jb1, so
            # the two copies overlap) and DMA it while the other half's
            # matmuls still run.
            out_sb = op.tile([T, HIDDEN], f32)
            half = len(pending_mm3) // 2
            order = sorted(range(len(pending_mm3)), key=lambda i: i % 2)
            for n, i in enumerate(order):
                pending_mm3[i]()
                if n == half - 1:
                    nc.scalar.copy(out_sb[:, 0:512], psum_out[:, 0:512])
                    nc.scalar.dma_start(out_d.ap()[:, 0:512],
                                        out_sb[:, 0:512])
            nc.vector.tensor_copy(out_sb[:, 512:1024], psum_out[:, 512:1024])
            nc.sync.dma_start(out_d.ap()[:, 512:1024],
                              out_sb[:, 512:1024])

    nc.compile()
    return nc


def _get_compiled():
    global _COMPILED
    if _COMPILED is None:
        _COMPILED = _build()
    return _COMPILED


def _pack_inputs(x, gate_proj, up_proj, down_proj):
    import ml_dtypes
    bf = ml_dtypes.bfloat16

    x = np.ascontiguousarray(x, dtype=np.float32)
    # xt[p, hc*T + t] = x[t, hc*128 + p]
    xt = np.ascontiguousarray(
        x.T.reshape(HC, P, T).transpose(1, 0, 2).reshape(P, HC * T)).astype(bf)
    eye = np.eye(T, dtype=bf)
    in_maps = []
    for k in range(N_CORES):
        g = np.asarray(gate_proj[k], dtype=np.float32)
        d = np.asarray(down_proj[k], dtype=np.float32)
        u = np.asarray(up_proj[k], dtype=np.float32)
        # wg4/wd4[c, p, hc, o] = g/d[c*CW + o, hc*128 + p]; interleave per hc
        wg4 = g.reshape(NCHUNK, CW, HC, P).transpose(0, 3, 2, 1)
        wd4 = d.reshape(NCHUNK, CW, HC, P).transpose(0, 3, 2, 1)
        wgd = np.concatenate([wg4, wd4], axis=3).reshape(NCHUNK, P, 2 * HC * CW)
        # Wu[c, p, f*HIDDEN + j] = u[j, c*CW + f*128 + p]
        wu = u.reshape(HIDDEN, NCHUNK, CW // P, P).transpose(1, 3, 2, 0).reshape(
            NCHUNK, P, 2 * HIDDEN)
        w = np.ascontiguousarray(
            np.concatenate([wgd, wu], axis=2)).astype(bf)
        in_maps.append({"xt": xt, "eye": eye, "w": w})
    return in_maps


def kernel(x, expert_indices, gate_proj, up_proj, down_proj):
    global LAST_RESULTS
    from concourse.bass_utils import run_bass_kernel_spmd

    nc = _get_compiled()
    in_maps = _pack_inputs(x, gate_proj, up_proj, down_proj)
    res = run_bass_kernel_spmd(nc, in_maps, core_ids=list(range(N_CORES)),
                               trace=TRACE)
    LAST_RESULTS = res

    expert_outs = np.stack([res.results[k]["out"] for k in range(N_CORES)])
    idx = np.asarray(expert_indices).astype(np.int64)  # [T, TOP_K]
    return expert_outs[idx, np.arange(T)[:, None], :].astype(np.float32)
